# revision 16
# baseline (speedup 1.0000x reference)
"""AutoCorrelationLoss Trainium2 kernel (8-core SPMD, data-parallel over batch).

Math: for each row x (length L=8192), with com = L - 128 = 8064 = 63*128:
  ac[k] = mean(x0c * (Y_k - mean(Y_k)))  where x0c = x[:com] - mean(x[:com])
Since sum(x0c) = 0 the mean(Y_k) term vanishes:
  com * ac[k] = c[k] = sum_j x0c[j] * x[j+k]
Decompose j = 128*t + p (t<63, p<128), T[t, f] = x[128t + f] (f<256),
Tc = T[:, :128] - mean(x[:com]).  With H = Tc.T @ T  ([128, 256]):
  c[k] = sum_j H[j, j+k]   (a skew-diagonal sum, k = 0..128)
r[k] = c[k]/c[0]; loss = mean_{b,k} |r_fake - r_real|.  r[0] == 1 on both
sides so the k=0 term contributes 0; c[0] (the divisor) is computed
directly as the elementwise dot <x0c, x[:com]> without the deskew.

Per core: 8 row-tensors (xin rows; 0-3 fake, 4-7 real).  Partition packing:
xin row 2i sits on partitions 0:63 (chunk index t = partition), row 2i+1 on
64:127 — engages both 8-port DMA halves and lets each H-matmul pair run in
separate PE row-groups (tile_position (0,0)/(64,0)) concurrently.

Pipeline (block pair i = xin rows {2i, 2i+1} = one deskew group):
  1. four strided loads (2 per HWDGE ring) -> xf fp32, block-pair pipelined
  2. per 2 blocks: 3D reduces -> block-diag ones-matmul -> negated means;
     centering fused with fp32->bf16 cast; raw stream cast to bf16
  3. c0 via fused mult+accum dots, summed by the same block-diag matmul;
     1/c0 (negated for real rows) broadcast to 128 partitions by rank-1
     matmuls -> bf16 rhs columns, all off the critical path
  4. per group: 2 concurrent H matmuls -> PSUM -> interleave-2 bf16 copy ->
     contiguous 128KB DRAM write -> single-DMA diagonal re-read
     R[j, 2k+u] = H_u[j, j+k] (128 x 516B descriptors); groups pipeline
     across both rings in alternating write/read directions
  5. c-extraction matmuls (lhsT = stride-2 R slice, rhs = +-1/c0 column)
     accumulate r_fake - r_real straight into a [128, 4] PSUM tile
  6. |.| reduce + ones-matmul partition sum -> out [1, 1] single-descriptor
Host sums 8 cores' scalars and divides by B*(NCOEF+1).
"""

import os
import sys

sys.path.insert(0, "/opt/trn_rl_repo")

import numpy as np

import concourse.bacc as bacc
import concourse.bass as bass
import concourse.mybir as mybir
import concourse.tile as tile
from concourse.bass_utils import run_bass_kernel_spmd
from concourse.tile_rust import add_dep_helper

B, L = 32, 8192
NCOEF = 128            # lags 0..128 -> 129 values
COM = L - NCOEF        # 8064 = 63 * 128
NT = 63                # contraction chunks
HALO = 256             # halo width per chunk
NK = NCOEF + 1         # 129
N_CORES = 8
ROWS_PER_CORE = B // N_CORES      # 4 batch rows per core
RT = 2 * ROWS_PER_CORE            # 8 row-tensors
NG = 4                            # deskew groups (2 rts each)
GW = 2 * HALO                     # group width (2 rts interleaved)

FP32 = mybir.dt.float32
BF16 = mybir.dt.bfloat16


def build_program():
    nc = bacc.Bacc(
        "TRN2",
        target_bir_lowering=False,
        debug=False,
        num_devices=N_CORES,
    )

    xin = nc.dram_tensor("xin", (RT, L), FP32, kind="ExternalInput")
    out = nc.dram_tensor("out", (1, 1), FP32, kind="ExternalOutput")

    with tile.TileContext(nc) as tc:
        with (
            tc.tile_pool(name="persist", bufs=1) as persist,
            tc.tile_pool(name="big", bufs=1) as bigp,
            tc.tile_pool(name="spool", bufs=1) as spool,
            tc.tile_pool(name="hdp", bufs=1, space=bass.MemorySpace.DRAM) as hdp,
            tc.tile_pool(name="hps", bufs=3, space=bass.MemorySpace.PSUM) as hps,
            tc.tile_pool(name="sps", bufs=1, space=bass.MemorySpace.PSUM) as sps,
        ):
            ones2b = persist.tile([65, 128], BF16)
            nc.gpsimd.memset(ones2b[:], 1.0)
            ones128f = persist.tile([128, 1], FP32)
            nc.gpsimd.memset(ones128f[:], 1.0)
            # block-diagonal ones: per-parity partition sums + broadcast
            w2 = persist.tile([128, 128], FP32)
            nc.gpsimd.memset(w2[:], 0.0)
            nc.gpsimd.memset(w2[0:NT, 0:NT], 1.0)
            nc.gpsimd.memset(w2[64:64 + NT, 64:64 + NT], 1.0)
            # parity totals onto partitions 0 (even) / 64 (odd)
            w4 = persist.tile([128, 65], FP32)
            nc.gpsimd.memset(w4[:], 0.0)
            nc.gpsimd.memset(w4[0:NT, 0:1], 1.0)
            nc.gpsimd.memset(w4[64:64 + NT, 64:65], 1.0)
            rowsum = spool.tile([128, NG], FP32, tag="rowsum")
            nc.gpsimd.memset(rowsum[:], 0.0)
            c0p = spool.tile([128, NG], FP32, tag="c0p")
            nc.gpsimd.memset(c0p[:], 0.0)

            # ---- 1. loads: xf[64e+t, i*256+c] = xin[2i+e, 128t+c] ----
            xf = bigp.tile([128, NG * HALO], FP32)
            lds = {}
            for bp in range(2):      # block pairs (0,1) then (2,3)
                cols = slice(bp * 2 * HALO, (bp + 1) * 2 * HALO)
                lds[(0, bp)] = nc.sync.dma_start(
                    xf[0:NT, cols],
                    bass.AP(xin, 4 * bp * L,
                            [[128, NT], [2 * L, 2], [1, HALO]]))
                lds[(1, bp)] = nc.scalar.dma_start(
                    xf[64:64 + NT, cols],
                    bass.AP(xin, (4 * bp + 1) * L,
                            [[128, NT], [2 * L, 2], [1, HALO]]))

            xs = bigp.tile([128, NG * HALO], BF16)
            wt = bigp.tile([128, NG * 128], BF16)
            junk = bigp.tile([128, 128], BF16)
            bcast = sps.tile([128, NG], FP32, tag="bcast")
            negm = spool.tile([128, NG], FP32, tag="negm")
            dps = sps.tile([128, RT], FP32, tag="dps")
            rbcps = sps.tile([128, RT], FP32, tag="rbcps")

            eng = {0: nc.vector, 1: nc.scalar}
            dot_ops = []
            group_state = {}

            for bp in range(2):
                cols = slice(bp * 2 * HALO, (bp + 1) * 2 * HALO)
                # ---- 2. stats + stream casts for this block pair ----
                for e, lo in ((0, 0), (1, 64)):
                    view = xf[lo:lo + NT, cols].rearrange(
                        "p (r c) -> p r c", r=2)[:, :, 0:128]
                    red = nc.vector.tensor_reduce(
                        rowsum[lo:lo + NT, 2 * bp:2 * bp + 2], view,
                        mybir.AxisListType.X, mybir.AluOpType.add)
                    add_dep_helper(red.ins, lds[(e, bp)].ins,
                                   reason="reduce reads xf view")
                    if e == 0:
                        cv = nc.vector.tensor_copy(xs[lo:lo + NT, cols],
                                                   xf[lo:lo + NT, cols])
                    else:
                        cv = nc.scalar.copy(xs[lo:lo + NT, cols],
                                            xf[lo:lo + NT, cols])
                    add_dep_helper(cv.ins, lds[(e, bp)].ins,
                                   reason="cast reads xf")
                nc.tensor.matmul(
                    bcast[:, 2 * bp:2 * bp + 2], w2[:],
                    rowsum[:, 2 * bp:2 * bp + 2], start=True, stop=True)
                nc.scalar.mul(negm[:, 2 * bp:2 * bp + 2],
                              bcast[:, 2 * bp:2 * bp + 2], -1.0 / COM)

                for g in (2 * bp, 2 * bp + 1):   # block = deskew group
                    # ---- 3a. centered weights (bf16) ----
                    wcols = slice(g * 128, (g + 1) * 128)
                    acols = slice(g * HALO, g * HALO + 128)
                    scol = negm[:, g:g + 1]
                    if g % 2 == 0:
                        nc.vector.tensor_scalar_add(
                            wt[:, wcols], xf[:, acols], scol)
                    else:
                        nc.scalar.add(wt[:, wcols], xf[:, acols], scol)
                    # ---- 3b. c0 partials: c0 = sum(x0c * x) = sum(x0c^2)
                    # (sum(x0c) == 0 over the com window)
                    for e, lo in ((0, 0), (1, 64)):
                        if (g + e) % 2 == 0:
                            dot = nc.vector.scalar_tensor_tensor(
                                junk[lo:lo + NT, :], wt[lo:lo + NT, wcols],
                                1.0, wt[lo:lo + NT, wcols],
                                mybir.AluOpType.bypass, mybir.AluOpType.mult,
                                accum_out=c0p[lo:lo + NT, g:g + 1])
                        else:
                            dot = nc.scalar.activation(
                                junk[lo:lo + NT, :], wt[lo:lo + NT, wcols],
                                mybir.ActivationFunctionType.Square,
                                accum_out=c0p[lo:lo + NT, g:g + 1])
                        dot_ops.append(dot)

                    # ---- 4. H matmul pair + interleave-2 copies ----
                    h_all = bigp.tile([128, GW], BF16, tag=f"hall{g}")
                    hv = h_all[:].rearrange("p (m u) -> p m u", u=2)
                    copies = []
                    for e, (lo, tp) in enumerate(((0, (0, 0)),
                                                  (64, (64, 0)))):
                        h_ps = hps.tile([128, HALO], FP32, tag="h")
                        nc.tensor.matmul(
                            h_ps[:], wt[lo:lo + NT, wcols],
                            xs[lo:lo + NT, g * HALO:(g + 1) * HALO],
                            start=True, stop=True, tile_position=tp)
                        cp = hv[:, :, e]
                        if e == 0:
                            copies.append(nc.vector.tensor_copy(cp, h_ps[:]))
                        else:
                            copies.append(nc.scalar.copy(cp, h_ps[:]))

                    hd = hdp.tile([128, GW], BF16, tag=f"hd{g}")
                    weng, reng = ((nc.sync, nc.scalar) if g % 2 == 0
                                  else (nc.scalar, nc.sync))
                    w = weng.dma_start(hd[:], h_all[:])
                    for cp_i in copies:
                        add_dep_helper(w.ins, cp_i.ins,
                                       reason="bounce write reads h_all view")
                    rbig = bigp.tile([128, 2 * NK], BF16, tag=f"rbig{g}")
                    diag = bass.AP(hd[:].tensor, 0,
                                   [[GW + 2, 128], [1, 2 * NK]])
                    r = reng.dma_start(rbig[:], diag)
                    add_dep_helper(r.ins, w.ins, reason="deskew reads hd")
                    group_state[g] = (rbig, r)

            # ---- 3c. 1/c0 columns (off critical path) ----
            c0t = sps.tile([65, NG], FP32, tag="c0t")
            mm0 = nc.tensor.matmul(c0t[:], w4[:], c0p[:],
                                   start=True, stop=True)
            for dot in dot_ops:
                add_dep_helper(mm0.ins, dot.ins, reason="c0 sums dots")
            rec = spool.tile([65, NG], BF16, tag="rec")
            with nc.allow_low_precision("bf16 1/c0 feeds bf16 matmuls; "
                                        "loss tolerance 2e-2"):
                nc.vector.reciprocal(rec[0:1, :], c0t[0:1, :])
                nc.vector.reciprocal(rec[64:65, :], c0t[64:65, :])
            # negate the real-row (blocks 2,3) reciprocals
            nc.vector.tensor_scalar_mul(rec[0:1, 2:4], rec[0:1, 2:4], -1.0)
            nc.vector.tensor_scalar_mul(rec[64:65, 2:4], rec[64:65, 2:4],
                                        -1.0)
            nc.tensor.matmul(rbcps[:, 0:NG], ones2b[0:1, :], rec[0:1, :],
                             start=True, stop=True)
            nc.tensor.matmul(rbcps[:, NG:RT], ones2b[64:65, :],
                             rec[64:65, :], start=True, stop=True)
            rbc_e = spool.tile([128, NG], BF16, tag="rbce")
            nc.vector.tensor_copy(rbc_e[:], rbcps[:, 0:NG])
            rbc_o = spool.tile([128, NG], BF16, tag="rbco")
            nc.scalar.copy(rbc_o[:], rbcps[:, NG:RT])

            # ---- 5. c matmuls: r columns (fake 0-3, real 4-7) ----
            for g in range(NG):
                rbig, r = group_state[g]
                rbv = rbig[:].rearrange("p (k u) -> p k u", u=2)
                for u in range(2):           # xin row = 2g + u
                    col = 2 * g + u
                    mm = nc.tensor.matmul(
                        dps[:, col:col + 1], rbv[:, 1:NK, u],
                        (rbc_e if u == 0 else rbc_o)[:, g:g + 1],
                        start=True, stop=True)
                    add_dep_helper(mm.ins, r.ins, reason="c mm reads rbig")

            # ---- 6. |r_f - r_r| -> partition sum -> scalar out ----
            rr_sb = spool.tile([128, NG], FP32, tag="rrsb")
            nc.scalar.copy(rr_sb[:], dps[:, NG:RT])
            dd = spool.tile([128, NG], FP32, tag="dd")
            nc.vector.tensor_add(dd[:], dps[:, 0:NG], rr_sb[:])
            absr = spool.tile([128, 1], FP32, tag="absr")
            nc.vector.tensor_reduce(
                absr[:], dd[:], mybir.AxisListType.X, mybir.AluOpType.add,
                apply_absolute_value=True)
            tps = sps.tile([1, 1], FP32, tag="tps")
            nc.tensor.matmul(tps[:], absr[:], ones128f[:],
                             start=True, stop=True)
            ts_sb = spool.tile([1, 1], FP32, tag="ts")
            nc.scalar.copy(ts_sb[:], tps[:])
            nc.sync.dma_start(out[:], ts_sb[:], single_packet=True)

    nc.compile()
    return nc


_CACHE = {}


def _get_program():
    if "nc" not in _CACHE:
        _CACHE["nc"] = build_program()
    return _CACHE["nc"]


def make_in_maps(fake: np.ndarray, real: np.ndarray):
    fake = np.asarray(fake, dtype=np.float32).reshape(B, L)
    real = np.asarray(real, dtype=np.float32).reshape(B, L)
    in_maps = []
    for c in range(N_CORES):
        rows = slice(c * ROWS_PER_CORE, (c + 1) * ROWS_PER_CORE)
        xin = np.concatenate([fake[rows], real[rows]], axis=0)
        in_maps.append({"xin": np.ascontiguousarray(xin)})
    return in_maps


def run(in_maps, **kwargs):
    """Run the SPMD program; returns (loss, BassKernelResults)."""
    res = run_bass_kernel_spmd(
        _get_program(), in_maps, list(range(N_CORES)), **kwargs
    )
    total = np.float64(0.0)
    for c in range(N_CORES):
        total += np.asarray(res.results[c]["out"], dtype=np.float64).sum()
    return np.float32(total / (B * NK)), res


def kernel(fake: np.ndarray, real: np.ndarray) -> np.ndarray:
    loss, _ = run(make_in_maps(fake, real))
    return loss
